# revision 2
# baseline (speedup 1.0000x reference)
"""Trainium2 Bass kernel for nn_ModalityPooling (segment attention-pooling).

Strategy (8 NeuronCores, SPMD):
  - Shard nodes of each modality into 8 contiguous ranges (data parallel);
    per-core ranges are padded with zero rows / batch-id 64 to a multiple of
    the 512-node macro tile so all cores run one identical program.
  - Single streaming pass over node features x (the memory-bound bulk):
    per 128-node subtile compute scorer s = relu(x@w1+b1)@w2+b2 on the
    tensor engine, e = exp(s) (no max subtraction needed: softmax weights
    e/sum(e) are invariant and |s| is tiny for this distribution), build
    masked weights W[n, b] = (batch[n]==b) * e[n] with one dual-op
    tensor_scalar against an iota tile, and accumulate
    num[b, :] += W.T @ [x | 1] into a persistent PSUM bank. Column 256 of
    num is the softmax denominator.
  - The scorer needs x feature-major; in the bf16 config that transpose is
    done by the DMA XBAR engine straight from HBM (dma_start_transpose), so
    the tensor engine only runs the three matmul stages.
  - Each core emits raw partials (64, 257) per modality; the host sums the
    8 partials (the cross-core "unshard"), normalizes pooled = num/den and
    applies the tiny (64x256) MLP heads in float32 numpy.
"""

import sys

import numpy as np

if "/opt/trn_rl_repo" not in sys.path:
    sys.path.insert(0, "/opt/trn_rl_repo")

import ml_dtypes  # noqa: E402

import concourse.bass as bass  # noqa: E402
import concourse.mybir as mybir  # noqa: E402
import concourse.tile as tile  # noqa: E402
from concourse import bacc  # noqa: E402
from concourse import bass_utils  # noqa: E402

F32 = mybir.dt.float32
BF16 = mybir.dt.bfloat16

NCORES = 8
D = 256
H = 64
B = 64
P = 128
MT = 512  # nodes per macro tile
SUB = MT // P  # 128-node subtiles per macro tile

# "bf16_xbar": x/w in bf16, scorer transpose via DMA XBAR  (fast path)
# "f32_pe":    everything f32, transposes on the tensor engine (accurate path)
CONFIG = "bf16_xbar"

# (tag, total nodes) per modality; shapes are hardcoded per the task contract.
MODS = [("g", 400000), ("c", 600000), ("r", 50000)]


def _per_core_padded(n_total: int) -> tuple[int, int]:
    per = n_total // NCORES
    assert per * NCORES == n_total
    padded = ((per + MT - 1) // MT) * MT
    return per, padded


def build_program(config: str):
    """Build the SPMD Bass program (identical for all 8 cores)."""
    bf16 = config == "bf16_xbar"
    XDT = BF16 if bf16 else F32

    nc = bacc.Bacc("TRN2", target_bir_lowering=False, debug=False,
                   num_devices=NCORES)

    dram_in = {}
    dram_out = {}
    dram_in["iota"] = nc.dram_tensor("iota", [P, H], F32, kind="ExternalInput")
    if not bf16:
        dram_in["ident"] = nc.dram_tensor("ident", [P, P], F32,
                                          kind="ExternalInput")
    for tag, n_total in MODS:
        _, npad = _per_core_padded(n_total)
        dram_in[f"x_{tag}"] = nc.dram_tensor(f"x_{tag}", [npad, D], XDT,
                                             kind="ExternalInput")
        dram_in[f"bf_{tag}"] = nc.dram_tensor(f"bf_{tag}", [npad], F32,
                                              kind="ExternalInput")
        dram_in[f"w1_{tag}"] = nc.dram_tensor(f"w1_{tag}", [D, H], XDT,
                                              kind="ExternalInput")
        dram_in[f"b1_{tag}"] = nc.dram_tensor(f"b1_{tag}", [H, 1], F32,
                                              kind="ExternalInput")
        dram_in[f"w2_{tag}"] = nc.dram_tensor(f"w2_{tag}", [H, 1], XDT,
                                              kind="ExternalInput")
        dram_in[f"b2_{tag}"] = nc.dram_tensor(f"b2_{tag}", [P, 1], F32,
                                              kind="ExternalInput")
        dram_out[f"out_{tag}"] = nc.dram_tensor(f"out_{tag}", [B, D + 1], F32,
                                                kind="ExternalOutput")

    with tile.TileContext(nc) as tc:
        with (
            tc.tile_pool(name="const", bufs=1) as cpool,
            tc.tile_pool(name="data", bufs=4) as dpool,
            tc.tile_pool(name="work", bufs=3) as wpool,
            tc.tile_pool(name="psum", bufs=2, space="PSUM") as ppool,
            tc.tile_pool(name="acc", bufs=1, space="PSUM") as apool,
        ):
            iota = cpool.tile([P, H], F32)
            nc.sync.dma_start(iota[:], dram_in["iota"].ap())
            if not bf16:
                ident = cpool.tile([P, P], F32)
                nc.sync.dma_start(ident[:], dram_in["ident"].ap())

            for tag, n_total in MODS:
                _, npad = _per_core_padded(n_total)
                nmac = npad // MT

                # Params
                w1_sb = cpool.tile([P, D // P, H], XDT, name=f"w1sb_{tag}")
                nc.sync.dma_start(
                    w1_sb[:],
                    dram_in[f"w1_{tag}"].ap().rearrange("(c p) j -> p c j", p=P),
                )
                b1_sb = cpool.tile([H, 1], F32, name=f"b1sb_{tag}")
                nc.sync.dma_start(b1_sb[:], dram_in[f"b1_{tag}"].ap())
                w2_sb = cpool.tile([H, 1], XDT, name=f"w2sb_{tag}")
                nc.sync.dma_start(w2_sb[:], dram_in[f"w2_{tag}"].ap())
                b2_sb = cpool.tile([P, 1], F32, name=f"b2sb_{tag}")
                nc.sync.dma_start(b2_sb[:], dram_in[f"b2_{tag}"].ap())

                # Whole (padded) batch-id array as f32, one column per subtile
                ncols = npad // P
                bf_sb = cpool.tile([P, ncols], F32, name=f"bfsb_{tag}")
                nc.sync.dma_start(
                    bf_sb[:], dram_in[f"bf_{tag}"].ap().rearrange("(t p) -> p t", p=P)
                )

                x_ap = dram_in[f"x_{tag}"].ap()
                x_r = x_ap.rearrange("(t j p) f -> t p j f", p=P, j=SUB)
                x_rows = x_ap.rearrange("(t n) f -> t n f", n=MT)

                num_ps = apool.tile([B, D + 1], F32, tag="num", name=f"num_{tag}")

                for t in range(nmac):
                    # Load 512 nodes; column 256 of each subtile row is 1.0
                    xe = dpool.tile([P, SUB, D + 1], XDT, tag="xe", name="xe")
                    nc.sync.dma_start(xe[:, :, 0:D], x_r[t])
                    nc.vector.memset(xe[:, :, D:D + 1], 1.0)

                    # x feature-major for the scorer
                    xt = wpool.tile([P, D // P, MT], XDT, tag="xt", name="xt")
                    if bf16:
                        for c in range(D // P):
                            nc.sync.dma_start_transpose(
                                xt[:, c, :], x_rows[t, :, c * P:(c + 1) * P]
                            )
                    else:
                        for c in range(D // P):
                            xt_ps = ppool.tile([P, MT], F32, tag="xt_ps",
                                               name="xt_ps")
                            for j in range(SUB):
                                nc.tensor.transpose(
                                    xt_ps[:, j * P:(j + 1) * P],
                                    xe[:, j, c * P:(c + 1) * P],
                                    ident,
                                )
                            nc.scalar.copy(xt[:, c, :], xt_ps[:])

                    # h^T = (x @ w1)^T : accumulate over the two 128-feat chunks
                    h_ps = ppool.tile([H, MT], F32, tag="h_ps", name="h_ps")
                    for c in range(D // P):
                        nc.tensor.matmul(h_ps[:], w1_sb[:, c, :], xt[:, c, :],
                                         start=(c == 0), stop=(c == D // P - 1))
                    hr = wpool.tile([H, MT], XDT, tag="hr", name="hr")
                    nc.scalar.activation(hr[:], h_ps[:],
                                         mybir.ActivationFunctionType.Relu,
                                         bias=b1_sb[:], scale=1.0)

                    # s (node-major): s[:, j] = hr_j^T @ w2   (128 nodes x 1)
                    s_ps = ppool.tile([P, SUB], F32, tag="s_ps", name="s_ps",
                                      bufs=1)
                    for j in range(SUB):
                        nc.tensor.matmul(s_ps[:, j:j + 1],
                                         hr[:, j * P:(j + 1) * P], w2_sb[:],
                                         start=True, stop=True)
                    e_sb = wpool.tile([P, SUB], F32, tag="e_sb", name="e_sb")
                    nc.scalar.activation(e_sb[:], s_ps[:],
                                         mybir.ActivationFunctionType.Exp,
                                         bias=b2_sb[:], scale=1.0)

                    # W[n, b] = (iota[b] == batch[n]) * e[n]
                    w_sb = wpool.tile([P, SUB, H], XDT, tag="w_sb", name="w_sb")
                    for j in range(SUB):
                        col = t * SUB + j
                        nc.vector.tensor_scalar(
                            out=w_sb[:, j, :],
                            in0=iota[:],
                            scalar1=bf_sb[:, col:col + 1],
                            scalar2=e_sb[:, j:j + 1],
                            op0=mybir.AluOpType.is_equal,
                            op1=mybir.AluOpType.mult,
                        )

                    # num += W^T @ [x | 1]
                    for j in range(SUB):
                        first = (t == 0 and j == 0)
                        last = (t == nmac - 1 and j == SUB - 1)
                        nc.tensor.matmul(num_ps[:], w_sb[:, j, :], xe[:, j, :],
                                         start=first, stop=last)

                out_sb = wpool.tile([B, D + 1], F32, tag="out_sb",
                                    name=f"outsb_{tag}")
                nc.scalar.copy(out_sb[:], num_ps[:])
                nc.sync.dma_start(dram_out[f"out_{tag}"].ap(), out_sb[:])

    nc.compile()
    return nc


def _prep_core_inputs(x_gene, x_cpg, x_mir, batch_gene, batch_cpg, batch_mir,
                      params, config: str):
    """Build the 8 per-core input maps (host-side shard + pad)."""
    bf16 = config == "bf16_xbar"
    xdt = ml_dtypes.bfloat16 if bf16 else np.float32

    iota = np.broadcast_to(np.arange(H, dtype=np.float32), (P, H)).copy()

    mod_data = {
        "g": (x_gene, batch_gene, params["pool_gene"]),
        "c": (x_cpg, batch_cpg, params["pool_cpg"]),
        "r": (x_mir, batch_mir, params["pool_mir"]),
    }

    common = {"iota": iota}
    if not bf16:
        common["ident"] = np.eye(P, dtype=np.float32)
    for tag, n_total in MODS:
        _, _, p = (*mod_data[tag],)
        w1, b1 = p["w1b1"]
        w2, b2 = p["w2b2"]
        common[f"w1_{tag}"] = np.ascontiguousarray(
            np.asarray(w1, dtype=np.float32).astype(xdt))
        common[f"b1_{tag}"] = np.ascontiguousarray(
            np.asarray(b1, dtype=np.float32).reshape(H, 1))
        common[f"w2_{tag}"] = np.ascontiguousarray(
            np.asarray(w2, dtype=np.float32).reshape(H, 1).astype(xdt))
        common[f"b2_{tag}"] = np.full(
            (P, 1), np.float32(np.asarray(b2).reshape(-1)[0]), dtype=np.float32)

    in_maps = []
    xcast = {}
    for tag, n_total in MODS:
        x, _, _ = mod_data[tag]
        xcast[tag] = np.asarray(x, dtype=np.float32).astype(xdt)
    for c in range(NCORES):
        m = dict(common)
        for tag, n_total in MODS:
            _, batch, _ = mod_data[tag]
            per, npad = _per_core_padded(n_total)
            st, en = c * per, (c + 1) * per
            xp = np.zeros((npad, D), dtype=xdt)
            xp[:per] = xcast[tag][st:en]
            bf = np.full((npad,), np.float32(B), dtype=np.float32)
            bf[:per] = np.asarray(batch[st:en]).astype(np.float32)
            m[f"x_{tag}"] = xp
            m[f"bf_{tag}"] = bf
        in_maps.append(m)
    return in_maps


def _finish_on_host(sums, params):
    """Normalize pooled vectors and apply the small linear heads (f32 numpy)."""
    pooled = {}
    for tag, _ in MODS:
        num = sums[tag]
        den = num[:, D:D + 1]
        with np.errstate(divide="ignore", invalid="ignore"):
            pool = np.where(den > 0, num[:, :D] / den, 0.0).astype(np.float32)
        pooled[tag] = pool

    def mlp_head(v, p):
        w1, b1 = p["w1b1"]
        w2, b2 = p["w2b2"]
        w1 = np.asarray(w1, np.float32)
        b1 = np.asarray(b1, np.float32)
        w2 = np.asarray(w2, np.float32)
        b2 = np.asarray(b2, np.float32)
        return np.maximum(v @ w1 + b1, 0.0) @ w2 + b2

    z_mrna = mlp_head(pooled["g"], params["mrna"]).astype(np.float32)
    z_cnv = mlp_head(pooled["g"], params["cnv"]).astype(np.float32)
    wc, bc = params["lin_cpg"]
    wm, bm = params["lin_mir"]
    z_dnam = (pooled["c"] @ np.asarray(wc, np.float32)
              + np.asarray(bc, np.float32)).astype(np.float32)
    z_mir = (pooled["r"] @ np.asarray(wm, np.float32)
             + np.asarray(bm, np.float32)).astype(np.float32)
    return (z_mrna, z_cnv, z_dnam, z_mir)


_PROGRAM_CACHE = {}


def run(x_gene, x_cpg, x_mir, batch_gene, batch_cpg, batch_mir, params,
        trace=False, trace_cores=None, config=None):
    """Run on 8 NeuronCores; returns (outputs_tuple, BassKernelResults)."""
    config = config or CONFIG
    if config not in _PROGRAM_CACHE:
        _PROGRAM_CACHE[config] = build_program(config)
    nc = _PROGRAM_CACHE[config]

    in_maps = _prep_core_inputs(x_gene, x_cpg, x_mir, batch_gene, batch_cpg,
                                batch_mir, params, config)
    kwargs = {}
    if trace:
        kwargs["trace"] = True
        if trace_cores is not None:
            kwargs["trace_cores"] = trace_cores
    res = bass_utils.run_bass_kernel_spmd(
        nc, in_maps, core_ids=list(range(NCORES)), **kwargs
    )

    sums = {}
    for tag, _ in MODS:
        acc = np.zeros((B, D + 1), dtype=np.float64)
        for c in range(NCORES):
            acc += res.results[c][f"out_{tag}"].astype(np.float64)
        sums[tag] = acc.astype(np.float32)

    out = _finish_on_host(sums, params)
    return out, res


def kernel(x_gene, x_cpg, x_mir, batch_gene, batch_cpg, batch_mir, params):
    out, _ = run(x_gene, x_cpg, x_mir, batch_gene, batch_cpg, batch_mir, params)
    return out


# revision 7
# speedup vs baseline: 1.6355x; 1.6355x over previous
"""Trainium2 Bass kernel for nn_ModalityPooling (segment attention-pooling).

Strategy (8 NeuronCores, SPMD):
  - Shard nodes of each modality into 8 contiguous ranges (data parallel);
    per-core ranges are padded with zero rows / batch-id 64 to a multiple of
    the 512-node macro tile so all cores run one identical program.
  - Single streaming pass over node features x (the memory-bound bulk):
    per 128-node subtile compute scorer s = relu(x@w1+b1)@w2+b2 on the
    tensor engine, e = exp(s) (no max subtraction needed: softmax weights
    e/sum(e) are invariant and |s| is tiny for this distribution), build
    masked weights W[n, b] = (batch[n]==b) * e[n] with one dual-op
    tensor_scalar against an iota tile, and accumulate
    num[b, :] += W.T @ [x | 1] into a persistent PSUM bank. Column 256 of
    num is the softmax denominator.
  - The scorer needs x feature-major; in the bf16 config that transpose is
    done by the DMA XBAR engine straight from HBM (dma_start_transpose), so
    the tensor engine only runs the three matmul stages.
  - Each core emits raw partials (64, 257) per modality; the host sums the
    8 partials (the cross-core "unshard"), normalizes pooled = num/den and
    applies the tiny (64x256) MLP heads in float32 numpy.
"""

import sys

import numpy as np

if "/opt/trn_rl_repo" not in sys.path:
    sys.path.insert(0, "/opt/trn_rl_repo")

import ml_dtypes  # noqa: E402

import concourse.bass as bass  # noqa: E402
import concourse.mybir as mybir  # noqa: E402
import concourse.tile as tile  # noqa: E402
from concourse import bacc  # noqa: E402
from concourse import bass_utils  # noqa: E402

F32 = mybir.dt.float32
BF16 = mybir.dt.bfloat16

NCORES = 8
D = 256
H = 64
B = 64
P = 128
MT = 512  # nodes per macro tile
SUB = MT // P  # 128-node subtiles per macro tile

# "bf16_xbar": x/w in bf16, scorer transpose via DMA XBAR  (fast path)
# "f32_pe":    everything f32, transposes on the tensor engine (accurate path)
CONFIG = "bf16_xbar"

# (tag, total nodes) per modality; shapes are hardcoded per the task contract.
MODS = [("g", 400000), ("c", 600000), ("r", 50000)]


def _per_core_padded(n_total: int) -> tuple[int, int]:
    per = n_total // NCORES
    assert per * NCORES == n_total
    padded = ((per + MT - 1) // MT) * MT
    return per, padded


def build_program(config: str):
    """Build the SPMD Bass program (identical for all 8 cores)."""
    bf16 = config == "bf16_xbar"
    XDT = BF16 if bf16 else F32

    nc = bacc.Bacc("TRN2", target_bir_lowering=False, debug=False,
                   num_devices=NCORES)

    dram_in = {}
    dram_out = {}
    dram_in["iota"] = nc.dram_tensor("iota", [P, H], F32, kind="ExternalInput")
    if not bf16:
        dram_in["ident"] = nc.dram_tensor("ident", [P, P], F32,
                                          kind="ExternalInput")
    for tag, n_total in MODS:
        _, npad = _per_core_padded(n_total)
        dram_in[f"x_{tag}"] = nc.dram_tensor(f"x_{tag}", [npad, D], XDT,
                                             kind="ExternalInput")
        dram_in[f"bf_{tag}"] = nc.dram_tensor(f"bf_{tag}", [npad], F32,
                                              kind="ExternalInput")
        dram_in[f"w1_{tag}"] = nc.dram_tensor(f"w1_{tag}", [D, H], XDT,
                                              kind="ExternalInput")
        dram_in[f"b1_{tag}"] = nc.dram_tensor(f"b1_{tag}", [H, 1], F32,
                                              kind="ExternalInput")
        dram_in[f"w2_{tag}"] = nc.dram_tensor(f"w2_{tag}", [H, 1], XDT,
                                              kind="ExternalInput")
        dram_in[f"b2_{tag}"] = nc.dram_tensor(f"b2_{tag}", [P, 1], F32,
                                              kind="ExternalInput")
        outp = 2 * B if bf16 else B  # bf16 path col-packs pooled into 128 rows
        dram_out[f"out_{tag}"] = nc.dram_tensor(f"out_{tag}", [outp, D + 1], F32,
                                                kind="ExternalOutput")

    with tile.TileContext(nc) as tc:
        with (
            tc.tile_pool(name="const", bufs=1) as cpool,
            tc.tile_pool(name="data", bufs=4) as dpool,
            tc.tile_pool(name="work", bufs=3) as wpool,
            tc.tile_pool(name="psum", bufs=2, space="PSUM") as ppool,
            tc.tile_pool(name="acc", bufs=1, space="PSUM") as apool,
        ):
            iota = cpool.tile([P, H], F32)
            nc.sync.dma_start(iota[:], dram_in["iota"].ap())
            if not bf16:
                ident = cpool.tile([P, P], F32)
                nc.sync.dma_start(ident[:], dram_in["ident"].ap())

            for tag, n_total in MODS:
                _, npad = _per_core_padded(n_total)
                nmac = npad // MT

                # Params
                w1_sb = cpool.tile([P, D // P, H], XDT, name=f"w1sb_{tag}")
                nc.sync.dma_start(
                    w1_sb[:],
                    dram_in[f"w1_{tag}"].ap().rearrange("(c p) j -> p c j", p=P),
                )
                b1_sb = cpool.tile([H, 1], F32, name=f"b1sb_{tag}")
                nc.sync.dma_start(b1_sb[:], dram_in[f"b1_{tag}"].ap())
                w2_sb = cpool.tile([H, 1], XDT, name=f"w2sb_{tag}")
                nc.sync.dma_start(w2_sb[:], dram_in[f"w2_{tag}"].ap())
                b2_sb = cpool.tile([P, 1], F32, name=f"b2sb_{tag}")
                nc.sync.dma_start(b2_sb[:], dram_in[f"b2_{tag}"].ap())

                # Whole (padded) batch-id array as f32, one column per subtile
                ncols = npad // P
                bf_sb = cpool.tile([P, ncols], F32, name=f"bfsb_{tag}")
                nc.sync.dma_start(
                    bf_sb[:], dram_in[f"bf_{tag}"].ap().rearrange("(t p) -> p t", p=P)
                )

                x_ap = dram_in[f"x_{tag}"].ap()
                x_r = x_ap.rearrange("(t j p) f -> t p j f", p=P, j=SUB)
                x_rows = x_ap.rearrange("(t n) f -> t n f", n=MT)

                outp = 2 * B if bf16 else B
                num_ps = apool.tile([outp, D + 1], F32, tag="num",
                                    name=f"num_{tag}")

                for t in range(nmac):
                    # Load 512 nodes; column 256 of each subtile row is 1.0
                    xe = dpool.tile([P, SUB, D + 1], XDT, tag="xe", name="xe")
                    if bf16:
                        # SWDGE ring: keeps the two HWDGE rings free for the
                        # XBAR transposes
                        nc.gpsimd.dma_start(xe[:, :, 0:D], x_r[t])
                    else:
                        nc.sync.dma_start(xe[:, :, 0:D], x_r[t])
                    nc.vector.memset(xe[:, :, D:D + 1], 1.0)

                    # x feature-major for the scorer
                    xt = wpool.tile([P, D // P, MT], XDT, tag="xt", name="xt")
                    if bf16:
                        nc.sync.dma_start_transpose(
                            xt[:, 0, :], x_rows[t, :, 0:P])
                        nc.scalar.dma_start_transpose(
                            xt[:, 1, :], x_rows[t, :, P:2 * P])
                    else:
                        for c in range(D // P):
                            xt_ps = ppool.tile([P, MT], F32, tag="xt_ps",
                                               name="xt_ps")
                            for j in range(SUB):
                                nc.tensor.transpose(
                                    xt_ps[:, j * P:(j + 1) * P],
                                    xe[:, j, c * P:(c + 1) * P],
                                    ident,
                                )
                            nc.scalar.copy(xt[:, c, :], xt_ps[:])

                    # h^T = (x @ w1)^T : accumulate over the two 128-feat chunks
                    h_ps = ppool.tile([H, MT], F32, tag="h_ps", name="h_ps")
                    for c in range(D // P):
                        nc.tensor.matmul(h_ps[:], w1_sb[:, c, :], xt[:, c, :],
                                         start=(c == 0), stop=(c == D // P - 1))
                    hr = wpool.tile([H, MT], XDT, tag="hr", name="hr")
                    if bf16:
                        # relu on DVE (one dual-op): hr = max(h + b1, 0)
                        nc.vector.tensor_scalar(
                            out=hr[:], in0=h_ps[:],
                            scalar1=b1_sb[:], scalar2=0.0,
                            op0=mybir.AluOpType.add,
                            op1=mybir.AluOpType.max,
                        )
                    else:
                        nc.scalar.activation(hr[:], h_ps[:],
                                             mybir.ActivationFunctionType.Relu,
                                             bias=b1_sb[:], scale=1.0)

                    # s (node-major): s[:, j] = hr_j^T @ w2   (128 nodes x 1)
                    s_ps = ppool.tile([P, SUB], F32, tag="s_ps", name="s_ps",
                                      bufs=1)
                    for j in range(SUB):
                        nc.tensor.matmul(s_ps[:, j:j + 1],
                                         hr[:, j * P:(j + 1) * P], w2_sb[:],
                                         start=True, stop=True)
                    e_sb = wpool.tile([P, SUB], F32, tag="e_sb", name="e_sb")
                    nc.scalar.activation(e_sb[:], s_ps[:],
                                         mybir.ActivationFunctionType.Exp,
                                         bias=b2_sb[:], scale=1.0)

                    # W[n, b] = (iota[b] == batch[n]) * e[n]
                    w_sb = wpool.tile([P, SUB, H], XDT, tag="w_sb", name="w_sb")
                    for j in range(SUB):
                        col = t * SUB + j
                        nc.vector.tensor_scalar(
                            out=w_sb[:, j, :],
                            in0=iota[:],
                            scalar1=bf_sb[:, col:col + 1],
                            scalar2=e_sb[:, j:j + 1],
                            op0=mybir.AluOpType.is_equal,
                            op1=mybir.AluOpType.mult,
                        )

                    # num += W^T @ [x | 1]; bf16 path col-packs subtile pairs
                    # into the upper/lower 64 PSUM partitions (concurrent MMs)
                    for j in range(SUB):
                        first = (t == 0 and j < 2)
                        last = (t == nmac - 1 and j >= SUB - 2)
                        if bf16:
                            half = j % 2
                            nc.tensor.matmul(
                                num_ps[half * B:(half + 1) * B, :],
                                w_sb[:, j, :], xe[:, j, :],
                                start=first, stop=last,
                                tile_position=(0, half * B),
                            )
                        else:
                            first = (t == 0 and j == 0)
                            last = (t == nmac - 1 and j == SUB - 1)
                            nc.tensor.matmul(num_ps[:], w_sb[:, j, :],
                                             xe[:, j, :],
                                             start=first, stop=last)

                out_sb = wpool.tile([outp, D + 1], F32, tag="out_sb",
                                    name=f"outsb_{tag}")
                nc.scalar.copy(out_sb[:], num_ps[:])
                nc.sync.dma_start(dram_out[f"out_{tag}"].ap(), out_sb[:])

    nc.compile()
    return nc


def _prep_core_inputs(x_gene, x_cpg, x_mir, batch_gene, batch_cpg, batch_mir,
                      params, config: str):
    """Build the 8 per-core input maps (host-side shard + pad)."""
    bf16 = config == "bf16_xbar"
    xdt = ml_dtypes.bfloat16 if bf16 else np.float32

    iota = np.broadcast_to(np.arange(H, dtype=np.float32), (P, H)).copy()

    mod_data = {
        "g": (x_gene, batch_gene, params["pool_gene"]),
        "c": (x_cpg, batch_cpg, params["pool_cpg"]),
        "r": (x_mir, batch_mir, params["pool_mir"]),
    }

    common = {"iota": iota}
    if not bf16:
        common["ident"] = np.eye(P, dtype=np.float32)
    for tag, n_total in MODS:
        _, _, p = (*mod_data[tag],)
        w1, b1 = p["w1b1"]
        w2, b2 = p["w2b2"]
        common[f"w1_{tag}"] = np.ascontiguousarray(
            np.asarray(w1, dtype=np.float32).astype(xdt))
        common[f"b1_{tag}"] = np.ascontiguousarray(
            np.asarray(b1, dtype=np.float32).reshape(H, 1))
        common[f"w2_{tag}"] = np.ascontiguousarray(
            np.asarray(w2, dtype=np.float32).reshape(H, 1).astype(xdt))
        common[f"b2_{tag}"] = np.full(
            (P, 1), np.float32(np.asarray(b2).reshape(-1)[0]), dtype=np.float32)

    in_maps = []
    xcast = {}
    for tag, n_total in MODS:
        x, _, _ = mod_data[tag]
        xcast[tag] = np.asarray(x, dtype=np.float32).astype(xdt)
    for c in range(NCORES):
        m = dict(common)
        for tag, n_total in MODS:
            _, batch, _ = mod_data[tag]
            per, npad = _per_core_padded(n_total)
            st, en = c * per, (c + 1) * per
            xp = np.zeros((npad, D), dtype=xdt)
            xp[:per] = xcast[tag][st:en]
            bf = np.full((npad,), np.float32(B), dtype=np.float32)
            bf[:per] = np.asarray(batch[st:en]).astype(np.float32)
            m[f"x_{tag}"] = xp
            m[f"bf_{tag}"] = bf
        in_maps.append(m)
    return in_maps


def _finish_on_host(sums, params):
    """Normalize pooled vectors and apply the small linear heads (f32 numpy)."""
    pooled = {}
    for tag, _ in MODS:
        num = sums[tag]
        den = num[:, D:D + 1]
        with np.errstate(divide="ignore", invalid="ignore"):
            pool = np.where(den > 0, num[:, :D] / den, 0.0).astype(np.float32)
        pooled[tag] = pool

    def mlp_head(v, p):
        w1, b1 = p["w1b1"]
        w2, b2 = p["w2b2"]
        w1 = np.asarray(w1, np.float32)
        b1 = np.asarray(b1, np.float32)
        w2 = np.asarray(w2, np.float32)
        b2 = np.asarray(b2, np.float32)
        return np.maximum(v @ w1 + b1, 0.0) @ w2 + b2

    z_mrna = mlp_head(pooled["g"], params["mrna"]).astype(np.float32)
    z_cnv = mlp_head(pooled["g"], params["cnv"]).astype(np.float32)
    wc, bc = params["lin_cpg"]
    wm, bm = params["lin_mir"]
    z_dnam = (pooled["c"] @ np.asarray(wc, np.float32)
              + np.asarray(bc, np.float32)).astype(np.float32)
    z_mir = (pooled["r"] @ np.asarray(wm, np.float32)
             + np.asarray(bm, np.float32)).astype(np.float32)
    return (z_mrna, z_cnv, z_dnam, z_mir)


_PROGRAM_CACHE = {}


def run(x_gene, x_cpg, x_mir, batch_gene, batch_cpg, batch_mir, params,
        trace=False, trace_cores=None, config=None):
    """Run on 8 NeuronCores; returns (outputs_tuple, BassKernelResults)."""
    config = config or CONFIG
    if config not in _PROGRAM_CACHE:
        _PROGRAM_CACHE[config] = build_program(config)
    nc = _PROGRAM_CACHE[config]

    in_maps = _prep_core_inputs(x_gene, x_cpg, x_mir, batch_gene, batch_cpg,
                                batch_mir, params, config)
    kwargs = {}
    if trace:
        kwargs["trace"] = True
        if trace_cores is not None:
            kwargs["trace_cores"] = trace_cores
    res = bass_utils.run_bass_kernel_spmd(
        nc, in_maps, core_ids=list(range(NCORES)), **kwargs
    )

    sums = {}
    for tag, _ in MODS:
        acc = np.zeros((B, D + 1), dtype=np.float64)
        for c in range(NCORES):
            part = res.results[c][f"out_{tag}"].astype(np.float64)
            if part.shape[0] == 2 * B:  # col-packed halves
                part = part[:B] + part[B:]
            acc += part
        sums[tag] = acc.astype(np.float32)

    out = _finish_on_host(sums, params)
    return out, res


def kernel(x_gene, x_cpg, x_mir, batch_gene, batch_cpg, batch_mir, params):
    out, _ = run(x_gene, x_cpg, x_mir, batch_gene, batch_cpg, batch_mir, params)
    return out


# revision 8
# speedup vs baseline: 2.2818x; 1.3952x over previous
"""Trainium2 Bass kernel for nn_ModalityPooling (segment attention-pooling).

Strategy (8 NeuronCores, SPMD):
  - Shard nodes of each modality into 8 contiguous ranges (data parallel);
    per-core ranges are padded with zero rows / batch-id 64 to a multiple of
    the macro tile so all cores run one identical program.
  - Single streaming pass over node features x (the memory-bound bulk):
    per 128-node subtile compute scorer s = relu(x@w1+b1)@w2+b2 on the
    tensor engine, e = exp(s) (no max subtraction needed: softmax weights
    e/sum(e) are invariant and |s| is tiny for this distribution), build
    masked weights W[n, b] = (batch[n]==b) * e[n] on the vector engine, and
    accumulate num[b, :] += W.T @ [x | 1] into a persistent PSUM bank.
    Column 256 of num is the softmax denominator.
  - The scorer needs x feature-major; in the bf16 config that transpose is
    done by the DMA XBAR engine straight from HBM (dma_start_transpose),
    split across the two HWDGE rings (sync + scalar), while the node-major
    load rides the GPSIMD SWDGE ring. The pooled matmuls are col-packed in
    pairs via tile_position so two run concurrently in the PE array.
  - Each core emits raw partials per modality; the host sums the 8 partials
    (and the two col-packed halves), normalizes pooled = num/den and applies
    the tiny (64x256) MLP heads in float32 numpy.
"""

import sys

import numpy as np

if "/opt/trn_rl_repo" not in sys.path:
    sys.path.insert(0, "/opt/trn_rl_repo")

import ml_dtypes  # noqa: E402

import concourse.bass as bass  # noqa: E402
import concourse.mybir as mybir  # noqa: E402
import concourse.tile as tile  # noqa: E402
from concourse import bacc  # noqa: E402
from concourse import bass_utils  # noqa: E402

F32 = mybir.dt.float32
BF16 = mybir.dt.bfloat16

NCORES = 8
D = 256
H = 64
B = 64
P = 128

# "bf16_xbar": x/w in bf16, scorer transpose via DMA XBAR  (fast path)
# "f32_pe":    everything f32, transposes on the tensor engine (accurate path)
CONFIG = "bf16_xbar"

# (tag, total nodes) per modality; shapes are hardcoded per the task contract.
MODS = [("g", 400000), ("c", 600000), ("r", 50000)]


def _macro(config: str) -> int:
    return 1024 if config == "bf16_xbar" else 512


def _per_core_padded(n_total: int, mt: int) -> tuple[int, int]:
    per = n_total // NCORES
    assert per * NCORES == n_total
    padded = ((per + mt - 1) // mt) * mt
    return per, padded


def build_program(config: str):
    """Build the SPMD Bass program (identical for all 8 cores)."""
    bf16 = config == "bf16_xbar"
    XDT = BF16 if bf16 else F32
    MT = _macro(config)
    SUB = MT // P

    nc = bacc.Bacc("TRN2", target_bir_lowering=False, debug=False,
                   num_devices=NCORES)

    dram_in = {}
    dram_out = {}
    dram_in["iota"] = nc.dram_tensor("iota", [P, H], F32, kind="ExternalInput")
    if not bf16:
        dram_in["ident"] = nc.dram_tensor("ident", [P, P], F32,
                                          kind="ExternalInput")
    for tag, n_total in MODS:
        _, npad = _per_core_padded(n_total, MT)
        dram_in[f"x_{tag}"] = nc.dram_tensor(f"x_{tag}", [npad, D], XDT,
                                             kind="ExternalInput")
        dram_in[f"bf_{tag}"] = nc.dram_tensor(f"bf_{tag}", [npad], F32,
                                              kind="ExternalInput")
        dram_in[f"w1_{tag}"] = nc.dram_tensor(f"w1_{tag}", [D, H], XDT,
                                              kind="ExternalInput")
        dram_in[f"b1_{tag}"] = nc.dram_tensor(f"b1_{tag}", [H, 1], F32,
                                              kind="ExternalInput")
        dram_in[f"w2_{tag}"] = nc.dram_tensor(f"w2_{tag}", [H, 1], XDT,
                                              kind="ExternalInput")
        dram_in[f"b2_{tag}"] = nc.dram_tensor(f"b2_{tag}", [P, 1], F32,
                                              kind="ExternalInput")
        outp = 2 * B if bf16 else B  # bf16 path col-packs pooled into 128 rows
        dram_out[f"out_{tag}"] = nc.dram_tensor(f"out_{tag}", [outp, D + 1], F32,
                                                kind="ExternalOutput")

    with tile.TileContext(nc) as tc:
        with (
            tc.tile_pool(name="const", bufs=1) as cpool,
            tc.tile_pool(name="data", bufs=4) as dpool,
            tc.tile_pool(name="work", bufs=4) as wpool,
            tc.tile_pool(name="psum", bufs=2, space="PSUM") as ppool,
            tc.tile_pool(name="acc", bufs=1, space="PSUM") as apool,
        ):
            iota = cpool.tile([P, H], F32)
            nc.sync.dma_start(iota[:], dram_in["iota"].ap())
            if not bf16:
                ident = cpool.tile([P, P], F32)
                nc.sync.dma_start(ident[:], dram_in["ident"].ap())

            for tag, n_total in MODS:
                _, npad = _per_core_padded(n_total, MT)
                nmac = npad // MT

                # Params
                w1_sb = cpool.tile([P, D // P, H], XDT, name=f"w1sb_{tag}")
                nc.sync.dma_start(
                    w1_sb[:],
                    dram_in[f"w1_{tag}"].ap().rearrange("(c p) j -> p c j", p=P),
                )
                b1_sb = cpool.tile([H, 1], F32, name=f"b1sb_{tag}")
                nc.sync.dma_start(b1_sb[:], dram_in[f"b1_{tag}"].ap())
                w2_sb = cpool.tile([H, 1], XDT, name=f"w2sb_{tag}")
                nc.sync.dma_start(w2_sb[:], dram_in[f"w2_{tag}"].ap())
                b2_sb = cpool.tile([P, 1], F32, name=f"b2sb_{tag}")
                nc.sync.dma_start(b2_sb[:], dram_in[f"b2_{tag}"].ap())

                # Whole (padded) batch-id array as f32, one column per subtile
                ncols = npad // P
                bf_sb = cpool.tile([P, ncols], F32, name=f"bfsb_{tag}")
                nc.sync.dma_start(
                    bf_sb[:], dram_in[f"bf_{tag}"].ap().rearrange("(t p) -> p t", p=P)
                )

                x_ap = dram_in[f"x_{tag}"].ap()
                x_r = x_ap.rearrange("(t j p) f -> t p j f", p=P, j=SUB)
                x_rows = x_ap.rearrange("(t n) f -> t n f", n=MT)

                outp = 2 * B if bf16 else B
                num_ps = apool.tile([outp, D + 1], F32, tag="num",
                                    name=f"num_{tag}")

                for t in range(nmac):
                    # Load MT nodes; column 256 of each subtile row is 1.0
                    xe = dpool.tile([P, SUB, D + 1], XDT, tag="xe", name="xe")
                    if bf16:
                        # SWDGE ring: keeps the two HWDGE rings free for the
                        # XBAR transposes
                        nc.gpsimd.dma_start(xe[:, :, 0:D], x_r[t])
                    else:
                        nc.sync.dma_start(xe[:, :, 0:D], x_r[t])
                    nc.vector.memset(xe[:, :, D:D + 1], 1.0)

                    # x feature-major for the scorer
                    xt = wpool.tile([P, D // P, MT], XDT, tag="xt", name="xt")
                    if bf16:
                        nc.sync.dma_start_transpose(
                            xt[:, 0, :], x_rows[t, :, 0:P])
                        nc.scalar.dma_start_transpose(
                            xt[:, 1, :], x_rows[t, :, P:2 * P])
                    else:
                        for c in range(D // P):
                            xt_ps = ppool.tile([P, MT], F32, tag="xt_ps",
                                               name="xt_ps")
                            for j in range(SUB):
                                nc.tensor.transpose(
                                    xt_ps[:, j * P:(j + 1) * P],
                                    xe[:, j, c * P:(c + 1) * P],
                                    ident,
                                )
                            nc.scalar.copy(xt[:, c, :], xt_ps[:])

                    # h^T = (x @ w1)^T : accumulate over the two 128-feat
                    # chunks; matmul N is capped at 512 by the PSUM bank, so
                    # wide macros write the psum in 512-wide halves.
                    NHALF = MT // 512
                    h_ps = ppool.tile([H, MT], F32, tag="h_ps", name="h_ps")
                    for half in range(NHALF):
                        sl = slice(half * 512, (half + 1) * 512)
                        for c in range(D // P):
                            nc.tensor.matmul(h_ps[:, sl], w1_sb[:, c, :],
                                             xt[:, c, sl],
                                             start=(c == 0),
                                             stop=(c == D // P - 1))
                    hr = wpool.tile([H, MT], XDT, tag="hr", name="hr")
                    if bf16:
                        # relu on DVE (one dual-op): hr = max(h + b1, 0)
                        nc.vector.tensor_scalar(
                            out=hr[:], in0=h_ps[:],
                            scalar1=b1_sb[:], scalar2=0.0,
                            op0=mybir.AluOpType.add,
                            op1=mybir.AluOpType.max,
                        )
                    else:
                        nc.scalar.activation(hr[:], h_ps[:],
                                             mybir.ActivationFunctionType.Relu,
                                             bias=b1_sb[:], scale=1.0)

                    # s (node-major): s[:, j] = hr_j^T @ w2   (128 nodes x 1)
                    s_ps = ppool.tile([P, SUB], F32, tag="s_ps", name="s_ps",
                                      bufs=2)
                    for j in range(SUB):
                        nc.tensor.matmul(s_ps[:, j:j + 1],
                                         hr[:, j * P:(j + 1) * P], w2_sb[:],
                                         start=True, stop=True)
                    e_sb = wpool.tile([P, SUB], F32, tag="e_sb", name="e_sb")
                    nc.scalar.activation(e_sb[:], s_ps[:],
                                         mybir.ActivationFunctionType.Exp,
                                         bias=b2_sb[:], scale=1.0)

                    # W[n, b] = (iota[b] == batch[n]) * e[n]
                    w_sb = wpool.tile([P, SUB, H], XDT, tag="w_sb", name="w_sb")
                    if bf16:
                        # two batched tensor_tensor ops with broadcast operands
                        cols = slice(t * SUB, (t + 1) * SUB)
                        nc.vector.tensor_tensor(
                            out=w_sb[:],
                            in0=iota[:, None, :].to_broadcast((P, SUB, H)),
                            in1=bf_sb[:, cols, None].to_broadcast((P, SUB, H)),
                            op=mybir.AluOpType.is_equal,
                        )
                        nc.vector.tensor_tensor(
                            out=w_sb[:], in0=w_sb[:],
                            in1=e_sb[:, :, None].to_broadcast((P, SUB, H)),
                            op=mybir.AluOpType.mult,
                        )
                    else:
                        for j in range(SUB):
                            col = t * SUB + j
                            nc.vector.tensor_scalar(
                                out=w_sb[:, j, :],
                                in0=iota[:],
                                scalar1=bf_sb[:, col:col + 1],
                                scalar2=e_sb[:, j:j + 1],
                                op0=mybir.AluOpType.is_equal,
                                op1=mybir.AluOpType.mult,
                            )

                    # num += W^T @ [x | 1]; bf16 path col-packs subtile pairs
                    # into the upper/lower 64 PSUM partitions (concurrent MMs)
                    for j in range(SUB):
                        if bf16:
                            first = (t == 0 and j < 2)
                            last = (t == nmac - 1 and j >= SUB - 2)
                            half = j % 2
                            nc.tensor.matmul(
                                num_ps[half * B:(half + 1) * B, :],
                                w_sb[:, j, :], xe[:, j, :],
                                start=first, stop=last,
                                tile_position=(0, half * B),
                            )
                        else:
                            first = (t == 0 and j == 0)
                            last = (t == nmac - 1 and j == SUB - 1)
                            nc.tensor.matmul(num_ps[:], w_sb[:, j, :],
                                             xe[:, j, :],
                                             start=first, stop=last)

                out_sb = wpool.tile([outp, D + 1], F32, tag="out_sb",
                                    name=f"outsb_{tag}")
                nc.scalar.copy(out_sb[:], num_ps[:])
                nc.sync.dma_start(dram_out[f"out_{tag}"].ap(), out_sb[:])

    nc.compile()
    return nc


def _prep_core_inputs(x_gene, x_cpg, x_mir, batch_gene, batch_cpg, batch_mir,
                      params, config: str):
    """Build the 8 per-core input maps (host-side shard + pad)."""
    bf16 = config == "bf16_xbar"
    xdt = ml_dtypes.bfloat16 if bf16 else np.float32
    MT = _macro(config)

    iota = np.broadcast_to(np.arange(H, dtype=np.float32), (P, H)).copy()

    mod_data = {
        "g": (x_gene, batch_gene, params["pool_gene"]),
        "c": (x_cpg, batch_cpg, params["pool_cpg"]),
        "r": (x_mir, batch_mir, params["pool_mir"]),
    }

    common = {"iota": iota}
    if not bf16:
        common["ident"] = np.eye(P, dtype=np.float32)
    for tag, n_total in MODS:
        p = mod_data[tag][2]
        w1, b1 = p["w1b1"]
        w2, b2 = p["w2b2"]
        common[f"w1_{tag}"] = np.ascontiguousarray(
            np.asarray(w1, dtype=np.float32).astype(xdt))
        common[f"b1_{tag}"] = np.ascontiguousarray(
            np.asarray(b1, dtype=np.float32).reshape(H, 1))
        common[f"w2_{tag}"] = np.ascontiguousarray(
            np.asarray(w2, dtype=np.float32).reshape(H, 1).astype(xdt))
        common[f"b2_{tag}"] = np.full(
            (P, 1), np.float32(np.asarray(b2).reshape(-1)[0]), dtype=np.float32)

    in_maps = []
    xcast = {}
    for tag, n_total in MODS:
        x = mod_data[tag][0]
        xcast[tag] = np.asarray(x, dtype=np.float32).astype(xdt)
    for c in range(NCORES):
        m = dict(common)
        for tag, n_total in MODS:
            batch = mod_data[tag][1]
            per, npad = _per_core_padded(n_total, MT)
            st, en = c * per, (c + 1) * per
            xp = np.zeros((npad, D), dtype=xdt)
            xp[:per] = xcast[tag][st:en]
            bf = np.full((npad,), np.float32(B), dtype=np.float32)
            bf[:per] = np.asarray(batch[st:en]).astype(np.float32)
            m[f"x_{tag}"] = xp
            m[f"bf_{tag}"] = bf
        in_maps.append(m)
    return in_maps


def _finish_on_host(sums, params):
    """Normalize pooled vectors and apply the small linear heads (f32 numpy)."""
    pooled = {}
    for tag, _ in MODS:
        num = sums[tag]
        den = num[:, D:D + 1]
        with np.errstate(divide="ignore", invalid="ignore"):
            pool = np.where(den > 0, num[:, :D] / den, 0.0).astype(np.float32)
        pooled[tag] = pool

    def mlp_head(v, p):
        w1, b1 = p["w1b1"]
        w2, b2 = p["w2b2"]
        w1 = np.asarray(w1, np.float32)
        b1 = np.asarray(b1, np.float32)
        w2 = np.asarray(w2, np.float32)
        b2 = np.asarray(b2, np.float32)
        return np.maximum(v @ w1 + b1, 0.0) @ w2 + b2

    z_mrna = mlp_head(pooled["g"], params["mrna"]).astype(np.float32)
    z_cnv = mlp_head(pooled["g"], params["cnv"]).astype(np.float32)
    wc, bc = params["lin_cpg"]
    wm, bm = params["lin_mir"]
    z_dnam = (pooled["c"] @ np.asarray(wc, np.float32)
              + np.asarray(bc, np.float32)).astype(np.float32)
    z_mir = (pooled["r"] @ np.asarray(wm, np.float32)
             + np.asarray(bm, np.float32)).astype(np.float32)
    return (z_mrna, z_cnv, z_dnam, z_mir)


_PROGRAM_CACHE = {}


def run(x_gene, x_cpg, x_mir, batch_gene, batch_cpg, batch_mir, params,
        trace=False, trace_cores=None, config=None):
    """Run on 8 NeuronCores; returns (outputs_tuple, BassKernelResults)."""
    config = config or CONFIG
    if config not in _PROGRAM_CACHE:
        _PROGRAM_CACHE[config] = build_program(config)
    nc = _PROGRAM_CACHE[config]

    in_maps = _prep_core_inputs(x_gene, x_cpg, x_mir, batch_gene, batch_cpg,
                                batch_mir, params, config)
    kwargs = {}
    if trace:
        kwargs["trace"] = True
        if trace_cores is not None:
            kwargs["trace_cores"] = trace_cores
    res = bass_utils.run_bass_kernel_spmd(
        nc, in_maps, core_ids=list(range(NCORES)), **kwargs
    )

    sums = {}
    for tag, _ in MODS:
        acc = np.zeros((B, D + 1), dtype=np.float64)
        for c in range(NCORES):
            part = res.results[c][f"out_{tag}"].astype(np.float64)
            if part.shape[0] == 2 * B:  # col-packed halves
                part = part[:B] + part[B:]
            acc += part
        sums[tag] = acc.astype(np.float32)

    out = _finish_on_host(sums, params)
    return out, res


def kernel(x_gene, x_cpg, x_mir, batch_gene, batch_cpg, batch_mir, params):
    out, _ = run(x_gene, x_cpg, x_mir, batch_gene, batch_cpg, batch_mir, params)
    return out
